# revision 1
# baseline (speedup 1.0000x reference)
"""Windowed cross-attention with relative position encodings, data-parallel
over batch across 8 NeuronCores.

Sharding (per spec hint): B=32 is split 4-per-core across the 8 cores;
the small q/kv/proj weights and the 169x1152 RPE table are replicated.
Windows are independent so attention needs no cross-device communication.

The RPE gather (static 169 -> [49,49] index table) is folded on the host
into dense per-(i,j,head) tables so each core runs pure einsum/softmax work.
"""

import functools

import numpy as np

import jax
import jax.numpy as jnp

WS = 7
NH = 12
DIM = 384
HD = DIM // NH
L = WS * WS
SCALE = HD ** (-0.5)
N_CORES = 8


def _relative_position_index() -> np.ndarray:
    coords = np.stack(np.meshgrid(np.arange(WS), np.arange(WS), indexing="ij"))
    flat = coords.reshape(2, -1)
    rel = flat[:, :, None] - flat[:, None, :]
    rel = rel.transpose(1, 2, 0).copy()
    rel[:, :, 0] += WS - 1
    rel[:, :, 1] += WS - 1
    rel[:, :, 0] *= 2 * WS - 1
    return rel.sum(-1)  # [L, L] int


_RPI = _relative_position_index()


def _partition(t, b, h, w):
    nh, nw = h // WS, w // WS
    t = t.reshape(b, nh, WS, nw, WS, NH, HD)
    t = t.transpose(0, 1, 3, 5, 2, 4, 6)
    return t.reshape(b * nh * nw, NH, L, HD)


def _unpartition(t, b, h, w):
    nh, nw = h // WS, w // WS
    t = t.reshape(b, nh, nw, NH, WS, WS, HD)
    t = t.transpose(0, 1, 4, 2, 5, 3, 6)
    return t.reshape(b, h, w, DIM)


def _core_fn(x, context, q_w, q_b, kv_w, kv_b, proj_w, proj_b,
             q_rpe, k_rpe, v_rpe):
    b, h, w, _ = x.shape
    q = x @ q_w + q_b
    kv = context @ kv_w + kv_b
    k, v = jnp.split(kv, 2, axis=-1)

    q = _partition(q, b, h, w) * SCALE
    k = _partition(k, b, h, w)
    v = _partition(v, b, h, w)

    qk = jnp.einsum("bhic,bhjc->bhij", q, k)
    qr = jnp.einsum("bhic,ijhc->bhij", q, k_rpe)
    kr = jnp.einsum("bhjc,ijhc->bhij", k, q_rpe)
    attn = jax.nn.softmax(qk + qr + kr, axis=-1)

    out = jnp.einsum("bhij,bhjc->bhic", attn, v) + jnp.einsum(
        "bhij,ijhc->bhic", attn, v_rpe
    )
    out = _unpartition(out, b, h, w)
    return out @ proj_w + proj_b


_PMAP = None


def _get_pmap():
    global _PMAP
    if _PMAP is None:
        _PMAP = jax.pmap(_core_fn, devices=jax.devices()[:N_CORES])
    return _PMAP


def _tile8(a):
    a = np.asarray(a)
    return np.broadcast_to(a, (N_CORES,) + a.shape)


def kernel(x, context, rpe_table, q_w, q_b, kv_w, kv_b, proj_w, proj_b):
    x = np.asarray(x)
    context = np.asarray(context)
    B, H, W, _ = x.shape
    per = B // N_CORES

    # host-side fold of the static gather: [169, 1152] -> three [L,L,NH,HD]
    rpe = np.asarray(rpe_table)[_RPI.reshape(-1)].reshape(L, L, NH, 3 * HD)
    q_rpe, k_rpe, v_rpe = np.split(rpe, 3, axis=-1)
    q_rpe = (q_rpe * SCALE).astype(np.float32)
    k_rpe = np.ascontiguousarray(k_rpe, dtype=np.float32)
    v_rpe = np.ascontiguousarray(v_rpe, dtype=np.float32)

    xs = x.reshape(N_CORES, per, H, W, DIM)
    cs = context.reshape(N_CORES, per, H, W, DIM)

    out = _get_pmap()(
        xs, cs,
        _tile8(q_w), _tile8(q_b),
        _tile8(kv_w), _tile8(kv_b),
        _tile8(proj_w), _tile8(proj_b),
        _tile8(q_rpe), _tile8(k_rpe), _tile8(v_rpe),
    )
    out = np.asarray(out).reshape(B, H, W, DIM)
    return out.astype(np.float32)



# revision 2
# speedup vs baseline: 1.1149x; 1.1149x over previous
"""Windowed cross-attention with relative position encodings, data-parallel
over batch across 8 NeuronCores.

Sharding (per spec hint): B=32 is split 4-per-core across the 8 cores; the
small q/kv/proj weights and RPE tables are replicated.  Windows are
independent so attention needs no cross-device communication.

The RPE gather (static 169 -> [49,49] index table) is folded on the host into
dense block-diagonal tables so the per-(i,j,head) contractions become 49
well-shaped batched matmuls instead of 588 tiny ones (the tiny-batch einsum
lowering is what made the original formulation pathologically slow on
NeuronCC).
"""

import numpy as np

import jax
import jax.numpy as jnp

WS = 7
NH = 12
DIM = 384
HD = DIM // NH
L = WS * WS
SCALE = HD ** (-0.5)
N_CORES = 8
NW = 64  # windows per 56x56 image (8x8 grid of 7x7)


def _relative_position_index() -> np.ndarray:
    coords = np.stack(np.meshgrid(np.arange(WS), np.arange(WS), indexing="ij"))
    flat = coords.reshape(2, -1)
    rel = flat[:, :, None] - flat[:, None, :]
    rel = rel.transpose(1, 2, 0).copy()
    rel[:, :, 0] += WS - 1
    rel[:, :, 1] += WS - 1
    rel[:, :, 0] *= 2 * WS - 1
    return rel.sum(-1)  # [L, L] int


_RPI = _relative_position_index()


def make_tables(rpe_table):
    """Fold the static RPE gather into block-diagonal matmul tables."""
    rpe = np.asarray(rpe_table)[_RPI.reshape(-1)].reshape(L, L, NH, 3 * HD)
    q_rpe, k_rpe, v_rpe = np.split(rpe, 3, axis=-1)  # each [i,j,h,c]
    q_rpe = q_rpe * SCALE
    kr_bd = np.zeros((L, NH * HD, NH * L), np.float32)  # [i, (h,c), (h,j)]
    qr_bd = np.zeros((L, NH * HD, NH * L), np.float32)  # [j, (h,c), (h,i)]
    vr_bd = np.zeros((L, NH * L, NH * HD), np.float32)  # [i, (h,j), (h,c)]
    for h in range(NH):
        kr_bd[:, h * HD:(h + 1) * HD, h * L:(h + 1) * L] = \
            k_rpe[:, :, h, :].transpose(0, 2, 1)
        qr_bd[:, h * HD:(h + 1) * HD, h * L:(h + 1) * L] = \
            q_rpe[:, :, h, :].transpose(1, 2, 0)
        vr_bd[:, h * L:(h + 1) * L, h * HD:(h + 1) * HD] = v_rpe[:, :, h, :]
    return kr_bd, qr_bd, vr_bd


def _core_fn(x, context, q_w, q_b, kv_w, kv_b, proj_w, proj_b,
             kr_bd, qr_bd, vr_bd):
    b, H, W, _ = x.shape
    n = b * H * W
    q = (x.reshape(n, DIM) @ q_w + q_b) * SCALE
    kv = context.reshape(n, DIM) @ kv_w + kv_b
    k = kv[:, :DIM]
    v = kv[:, DIM:]

    def part(t):
        # [b,H,W,D] -> [b*NW, L, D] window-major
        t = t.reshape(b, 8, WS, 8, WS, DIM)
        t = t.transpose(0, 1, 3, 2, 4, 5)
        return t.reshape(b * NW, L, DIM)

    q = part(q.reshape(b, H, W, DIM))
    k = part(k.reshape(b, H, W, DIM))
    v = part(v.reshape(b, H, W, DIM))
    Bw = b * NW

    qh = q.reshape(Bw, L, NH, HD).transpose(0, 2, 1, 3)   # [Bw,NH,L,HD]
    kh = k.reshape(Bw, L, NH, HD).transpose(0, 2, 3, 1)   # [Bw,NH,HD,L]
    qk = qh @ kh                                          # [Bw,NH,i,j]

    q_t = q.transpose(1, 0, 2)                            # [i, Bw, D]
    qr = q_t @ kr_bd                                      # [i, Bw, (h,j)]
    qr = qr.reshape(L, Bw, NH, L).transpose(1, 2, 0, 3)   # [Bw,NH,i,j]

    k_t = k.transpose(1, 0, 2)                            # [j, Bw, D]
    kr = k_t @ qr_bd                                      # [j, Bw, (h,i)]
    kr = kr.reshape(L, Bw, NH, L).transpose(1, 2, 3, 0)   # [Bw,NH,i,j]

    attn = jax.nn.softmax(qk + qr + kr, axis=-1)

    vh = v.reshape(Bw, L, NH, HD).transpose(0, 2, 1, 3)   # [Bw,NH,L,HD]
    out1 = attn @ vh                                      # [Bw,NH,i,HD]

    at = attn.transpose(2, 0, 1, 3).reshape(L, Bw, NH * L)
    out2 = at @ vr_bd                                     # [i, Bw, (h,c)]
    out2 = out2.reshape(L, Bw, NH, HD).transpose(1, 2, 0, 3)

    out = (out1 + out2).transpose(0, 2, 1, 3).reshape(Bw, L, DIM)
    out = out.reshape(b, 8, 8, WS, WS, DIM).transpose(0, 1, 3, 2, 4, 5)
    out = out.reshape(n, DIM)
    return (out @ proj_w + proj_b).reshape(b, H, W, DIM)


_PMAP = None


def _get_pmap():
    global _PMAP
    if _PMAP is None:
        _PMAP = jax.pmap(_core_fn, devices=jax.devices()[:N_CORES])
    return _PMAP


def _tile8(a):
    a = np.asarray(a)
    return np.broadcast_to(a, (N_CORES,) + a.shape)


def prepare_args(x, context, rpe_table, q_w, q_b, kv_w, kv_b,
                 proj_w, proj_b):
    x = np.asarray(x)
    context = np.asarray(context)
    B, H, W, _ = x.shape
    per = B // N_CORES
    kr_bd, qr_bd, vr_bd = make_tables(rpe_table)
    return [
        x.reshape(N_CORES, per, H, W, DIM),
        context.reshape(N_CORES, per, H, W, DIM),
        _tile8(q_w), _tile8(q_b),
        _tile8(kv_w), _tile8(kv_b),
        _tile8(proj_w), _tile8(proj_b),
        _tile8(kr_bd), _tile8(qr_bd), _tile8(vr_bd),
    ]


def kernel(x, context, rpe_table, q_w, q_b, kv_w, kv_b, proj_w, proj_b):
    B, H, W, _ = np.asarray(x).shape
    args = prepare_args(x, context, rpe_table, q_w, q_b, kv_w, kv_b,
                        proj_w, proj_b)
    out = _get_pmap()(*args)
    return np.asarray(out).reshape(B, H, W, DIM).astype(np.float32)


# revision 4
# speedup vs baseline: 1.5881x; 1.4245x over previous
"""Windowed cross-attention with relative position encodings, data-parallel
over batch across 8 NeuronCores.

Sharding (per spec hint): B=32 is split 4-per-core across the 8 cores; the
small q/kv/proj weights and RPE tables are replicated.  Windows are
independent so attention needs no cross-device communication.

The RPE gather (static 169 -> [49,49] index table) is folded on the host into
dense block-diagonal tables so the per-(i,j,head) contractions become 49
well-shaped batched matmuls instead of 588 tiny ones (the tiny-batch einsum
lowering is what made the original formulation pathologically slow on
NeuronCC).
"""

import numpy as np

import jax
import jax.numpy as jnp

WS = 7
NH = 12
DIM = 384
HD = DIM // NH
L = WS * WS
SCALE = HD ** (-0.5)
N_CORES = 8
NW = 64  # windows per 56x56 image (8x8 grid of 7x7)


def _relative_position_index() -> np.ndarray:
    coords = np.stack(np.meshgrid(np.arange(WS), np.arange(WS), indexing="ij"))
    flat = coords.reshape(2, -1)
    rel = flat[:, :, None] - flat[:, None, :]
    rel = rel.transpose(1, 2, 0).copy()
    rel[:, :, 0] += WS - 1
    rel[:, :, 1] += WS - 1
    rel[:, :, 0] *= 2 * WS - 1
    return rel.sum(-1)  # [L, L] int


_RPI = _relative_position_index()


def make_tables(rpe_table):
    """Fold the static RPE gather into block-diagonal matmul tables."""
    rpe = np.asarray(rpe_table)[_RPI.reshape(-1)].reshape(L, L, NH, 3 * HD)
    q_rpe, k_rpe, v_rpe = np.split(rpe, 3, axis=-1)  # each [i,j,h,c]
    q_rpe = q_rpe * SCALE
    kr_bd = np.zeros((L, NH * HD, NH * L), np.float32)  # [i, (h,c), (h,j)]
    qr_bd = np.zeros((L, NH * HD, NH * L), np.float32)  # [j, (h,c), (h,i)]
    vr_bd = np.zeros((L, NH * L, NH * HD), np.float32)  # [i, (h,j), (h,c)]
    for h in range(NH):
        kr_bd[:, h * HD:(h + 1) * HD, h * L:(h + 1) * L] = \
            k_rpe[:, :, h, :].transpose(0, 2, 1)
        qr_bd[:, h * HD:(h + 1) * HD, h * L:(h + 1) * L] = \
            q_rpe[:, :, h, :].transpose(1, 2, 0)
        vr_bd[:, h * L:(h + 1) * L, h * HD:(h + 1) * HD] = v_rpe[:, :, h, :]
    return kr_bd, qr_bd, vr_bd


BF = jnp.bfloat16
F32 = jnp.float32


def _mm(a, b):
    # batched matmul with fp32 accumulation (bf16 inputs)
    return jax.lax.dot_general(
        a, b, (((a.ndim - 1,), (b.ndim - 2,)),
               (tuple(range(a.ndim - 2)), tuple(range(b.ndim - 2)))),
        preferred_element_type=F32)


def _core_fn(x, context, q_w, q_b, kv_w, kv_b, proj_w, proj_b,
             kr_bd, qr_bd, vr_bd):
    b, H, W, _ = x.shape
    n = b * H * W
    x = x.astype(BF)
    context = context.astype(BF)
    q_w = q_w.astype(BF)
    kv_w = kv_w.astype(BF)
    kr_bd = kr_bd.astype(BF)
    qr_bd = qr_bd.astype(BF)
    vr_bd = vr_bd.astype(BF)
    q = ((_mm(x.reshape(n, DIM), q_w) + q_b) * SCALE).astype(BF)
    kv = _mm(context.reshape(n, DIM), kv_w) + kv_b
    k = kv[:, :DIM].astype(BF)
    v = kv[:, DIM:].astype(BF)

    def part(t):
        # [b,H,W,D] -> [b*NW, L, D] window-major
        t = t.reshape(b, 8, WS, 8, WS, DIM)
        t = t.transpose(0, 1, 3, 2, 4, 5)
        return t.reshape(b * NW, L, DIM)

    q = part(q.reshape(b, H, W, DIM))
    k = part(k.reshape(b, H, W, DIM))
    v = part(v.reshape(b, H, W, DIM))
    Bw = b * NW

    qh = q.reshape(Bw, L, NH, HD).transpose(0, 2, 1, 3)   # [Bw,NH,L,HD]
    kh = k.reshape(Bw, L, NH, HD).transpose(0, 2, 3, 1)   # [Bw,NH,HD,L]
    qk = _mm(qh, kh)                                          # [Bw,NH,i,j]

    q_t = q.transpose(1, 0, 2)                            # [i, Bw, D]
    qr = _mm(q_t, kr_bd)                                      # [i, Bw, (h,j)]
    qr = qr.reshape(L, Bw, NH, L).transpose(1, 2, 0, 3)   # [Bw,NH,i,j]

    k_t = k.transpose(1, 0, 2)                            # [j, Bw, D]
    kr = _mm(k_t, qr_bd)                                      # [j, Bw, (h,i)]
    kr = kr.reshape(L, Bw, NH, L).transpose(1, 2, 3, 0)   # [Bw,NH,i,j]

    attn = jax.nn.softmax(qk + qr + kr, axis=-1).astype(BF)

    vh = v.reshape(Bw, L, NH, HD).transpose(0, 2, 1, 3)   # [Bw,NH,L,HD]
    out1 = _mm(attn, vh)                                      # [Bw,NH,i,HD]

    at = attn.transpose(2, 0, 1, 3).reshape(L, Bw, NH * L)
    out2 = _mm(at, vr_bd)                                     # [i, Bw, (h,c)]
    out2 = out2.reshape(L, Bw, NH, HD).transpose(1, 2, 0, 3)

    out = (out1 + out2).astype(BF).transpose(0, 2, 1, 3).reshape(Bw, L, DIM)
    out = out.reshape(b, 8, 8, WS, WS, DIM).transpose(0, 1, 3, 2, 4, 5)
    out = out.reshape(n, DIM)
    return (_mm(out, proj_w.astype(BF)) + proj_b).reshape(b, H, W, DIM)


_PMAP = None


def _get_pmap():
    global _PMAP
    if _PMAP is None:
        _PMAP = jax.pmap(_core_fn, devices=jax.devices()[:N_CORES])
    return _PMAP


def _tile8(a):
    a = np.asarray(a)
    return np.broadcast_to(a, (N_CORES,) + a.shape)


def prepare_args(x, context, rpe_table, q_w, q_b, kv_w, kv_b,
                 proj_w, proj_b):
    x = np.asarray(x)
    context = np.asarray(context)
    B, H, W, _ = x.shape
    per = B // N_CORES
    kr_bd, qr_bd, vr_bd = make_tables(rpe_table)
    return [
        x.reshape(N_CORES, per, H, W, DIM),
        context.reshape(N_CORES, per, H, W, DIM),
        _tile8(q_w), _tile8(q_b),
        _tile8(kv_w), _tile8(kv_b),
        _tile8(proj_w), _tile8(proj_b),
        _tile8(kr_bd), _tile8(qr_bd), _tile8(vr_bd),
    ]


def kernel(x, context, rpe_table, q_w, q_b, kv_w, kv_b, proj_w, proj_b):
    B, H, W, _ = np.asarray(x).shape
    args = prepare_args(x, context, rpe_table, q_w, q_b, kv_w, kv_b,
                        proj_w, proj_b)
    out = _get_pmap()(*args)
    return np.asarray(out).reshape(B, H, W, DIM).astype(np.float32)
